# revision 9
# baseline (speedup 1.0000x reference)
"""AttMatrixCov loss kernel for 8 Trainium2 NeuronCores.

Math
----
Reference:
    loss = sum_{a, i<j} mean((attc[a,i] outer attc[a,j] - I_C)^2)
         + sum_{a, i<j} mean((atts[a,i]^T atts[a,j] - I_W)^2)

Using sum_{c,d}(x_c y_d - d_cd)^2 = |x|^2 |y|^2 - 2 x.y + C and
|S_i^T S_j|_F^2 = <A_i, A_j> with A_t = S_t S_t^T, every pairwise sum
collapses via sum_{i<j} <u_i, u_j> = 1/2 (|sum_t u_t|^2 - sum_t |u_t|^2):

    loss_c[a] = ( 1/2((sum_t n_t)^2 - sum_t n_t^2)
                - (|sum_t c_t|^2 - sum_t n_t) + P*C ) / C^2,  n_t = |attc[a,t]|^2
    loss_s[a] = ( 1/2(|M_a|_F^2 - sum_t |A_t|_F^2)
                - (|R_a|_F^2 - sum_t |S_t|_F^2) + P*W ) / W^2,
                M_a = sum_t A_t,  R_a = sum_t S_t,  P = 28 pairs.

Sharding: 8 cores = (natt=4) x (ntemp halves=2). Each core loads its 4
atts slices (1 MB f32) plus a bf16 copy (0.5 MB) that feeds the PE:
DMA-transpose loads give S^T directly, so the PE only runs the 16
A_t = S_t S_t^T matmuls (bf16 operands, f32 PSUM accumulate; measured
end-to-end rel err ~1e-5). R, |S|^2 and the channel branch stay exact
f32. Host combines per-core partials (cross-half |M1+M2|^2 etc.).
"""

import numpy as np

NATT, NTEMP, C = 4, 8, 1024
H, W = 256, 256
TL = NTEMP // 2          # ntemp slices per core
NPAIR = NTEMP * (NTEMP - 1) // 2
P = 128
N_CORES = 8

_nc_cache = None


def _build():
    import concourse.bacc as bacc
    import concourse.tile as tile
    from concourse import mybir

    f32 = mybir.dt.float32
    bf16 = mybir.dt.bfloat16
    nc = bacc.Bacc()
    s_in = nc.dram_tensor("s", [TL, H, W], f32, kind="ExternalInput")
    sb_in = nc.dram_tensor("sb", [TL, H, W], bf16, kind="ExternalInput")
    c_in = nc.dram_tensor("c", [TL, C], f32, kind="ExternalInput")
    m_out = nc.dram_tensor("m_out", [P, 2, H], f32, kind="ExternalOutput")
    r_out = nc.dram_tensor("r_out", [H, W], f32, kind="ExternalOutput")
    st_out = nc.dram_tensor("stats", [P, 18], f32, kind="ExternalOutput")

    with tile.TileContext(nc) as tc:
        with (
            tc.tile_pool(name="sall", bufs=1) as sall,
            tc.tile_pool(name="htp", bufs=4) as htp,
            tc.tile_pool(name="acc", bufs=1) as accp,
            tc.tile_pool(name="scr", bufs=2) as scr,
            tc.tile_pool(name="ps_a", bufs=2, space="PSUM") as ps_a,
        ):
            # ---- loads ----
            s_all = sall.tile([P, TL, 2, W], f32)  # [p, t, hb, w]
            for t in range(TL):
                nc.gpsimd.dma_start(
                    out=s_all[:, t],
                    in_=s_in[t].rearrange("(hb p) w -> p hb w", p=P),
                )
            hts = []
            for t in range(TL):
                ht = htp.tile([P, 2, H], bf16)  # S^T (w split over (wb,p))
                nc.sync.dma_start(out=ht, in_=sb_in[t][:, :], transpose=True)
                hts.append(ht)
            c3 = sall.tile([P, TL, 8], f32)  # c3[p,t,f] = attc[t, p*8+f]
            nc.sync.dma_start(out=c3, in_=c_in.rearrange("t (p f) -> p t f", p=P))

            m_acc = accp.tile([P, 2, H], f32)
            r_acc = accp.tile([P, 2, W], f32)
            stats_a = accp.tile([P, TL], f32)   # ACT-written: |A_t|^2 partials
            stats_d = accp.tile([P, 14], f32)
            # stats_d cols: 0:2 |S|^2 half-partials, 2:6 n_t, 6:14 v

            # ---- channel branch (gpsimd, exact f32) ----
            csq = scr.tile([P, TL, 8], f32)
            nc.gpsimd.tensor_mul(csq, c3, c3)
            nc.vector.reduce_sum(
                stats_d[:, 2:6].rearrange("p (a b) -> p a b", b=1),
                csq,
                axis=mybir.AxisListType.X,
            )
            vtmp = scr.tile([P, 2, 8], f32)
            nc.gpsimd.tensor_add(vtmp, c3[:, 0:2, :], c3[:, 2:4, :])
            nc.gpsimd.tensor_add(stats_d[:, 6:14], vtmp[:, 0, :], vtmp[:, 1, :])

            # ---- A_t matmuls + M/sumA ----
            for t in range(TL):
                ht = hts[t]
                psa = ps_a.tile([P, 2, H], f32)
                for m in range(2):
                    for k in range(2):
                        nc.tensor.matmul(
                            psa[:, m, :],
                            lhsT=ht[:, k, m * P : (m + 1) * P],
                            rhs=ht[:, k, :],
                            start=(k == 0),
                            stop=(k == 1),
                        )
                a_scr = scr.tile([P, 2, H], f32, tag="ascr")
                nc.scalar.activation(
                    out=a_scr,
                    in_=psa,
                    func=mybir.ActivationFunctionType.Square,
                    accum_out=stats_a[:, t : t + 1],
                )
                if t == 0:
                    nc.vector.tensor_copy(m_acc, psa)
                else:
                    nc.vector.tensor_add(m_acc, m_acc, psa)

            # ---- R tree + |S|^2 (DVE, exact f32) ----
            r23 = scr.tile([P, 2, W], f32)
            nc.vector.tensor_add(r_acc, s_all[:, 0], s_all[:, 1])
            nc.vector.tensor_add(r23, s_all[:, 2], s_all[:, 3])
            nc.vector.tensor_add(r_acc, r_acc, r23)
            for hhalf in range(2):
                ssq = scr.tile([P, 2, 2, W], f32, tag="ssq")
                nc.vector.scalar_tensor_tensor(
                    out=ssq,
                    in0=s_all[:, 2 * hhalf : 2 * hhalf + 2],
                    scalar=1.0,
                    in1=s_all[:, 2 * hhalf : 2 * hhalf + 2],
                    op0=mybir.AluOpType.bypass,
                    op1=mybir.AluOpType.mult,
                    accum_out=stats_d[:, hhalf : hhalf + 1],
                )

            # ---- outputs ----
            nc.sync.dma_start(out=m_out[:, :, :], in_=m_acc)
            nc.sync.dma_start(
                out=r_out.rearrange("(hb p) w -> p hb w", p=P), in_=r_acc
            )
            nc.sync.dma_start(out=st_out[:, 0:4], in_=stats_a)
            nc.sync.dma_start(out=st_out[:, 4:18], in_=stats_d)
    nc.finalize()
    return nc


last_results = None


def _ensure_ntff_hook():
    """Register the axon NTFF profile hook if the image's antenv lacks it.

    Only matters when BASS_TRACE=1; harmless otherwise."""
    import sys
    import types

    try:
        import antenv.axon_hooks  # noqa: F401

        return
    except ImportError:
        pass
    try:
        from trn_agent_boot.trn_boot import _ntff_profile_via_ctypes

        hook = _ntff_profile_via_ctypes("/opt/axon/libaxon_pjrt.so")
    except Exception:
        hook = None
    mod = types.ModuleType("antenv.axon_hooks")
    mod.get_axon_ntff_profile_hook = lambda: hook
    mod.set_axon_ntff_profile_hook = lambda h: None
    sys.modules["antenv.axon_hooks"] = mod


def kernel(attc: np.ndarray, atts: np.ndarray) -> np.ndarray:
    global _nc_cache, last_results
    _ensure_ntff_hook()
    import ml_dtypes
    from concourse.bass_utils import run_bass_kernel_spmd

    if _nc_cache is None:
        _nc_cache = _build()
    nc = _nc_cache

    in_maps = []
    for core in range(N_CORES):
        a, hhalf = core // 2, core % 2
        sl = slice(hhalf * TL, (hhalf + 1) * TL)
        s_shard = np.ascontiguousarray(atts[a, sl]).astype(np.float32)
        in_maps.append(
            {
                "s": s_shard,
                "sb": s_shard.astype(ml_dtypes.bfloat16),
                "c": np.ascontiguousarray(attc[a, sl]).astype(np.float32),
            }
        )

    res = run_bass_kernel_spmd(nc, in_maps, core_ids=list(range(N_CORES)))
    last_results = res
    outs = res.results

    total = 0.0
    for a in range(NATT):
        o1, o2 = outs[2 * a], outs[2 * a + 1]
        mo = o1["m_out"].astype(np.float64) + o2["m_out"].astype(np.float64)
        M = mo.transpose(1, 0, 2).reshape(H, H)  # rows m*128+p
        R = o1["r_out"].astype(np.float64) + o2["r_out"].astype(np.float64)
        st1 = o1["stats"].astype(np.float64)
        st2 = o2["stats"].astype(np.float64)
        sumA = st1[:, 0:4].sum() + st2[:, 0:4].sum()
        sumS = st1[:, 4:6].sum() + st2[:, 4:6].sum()
        n_t = np.concatenate([st1[:, 6:10].sum(0), st2[:, 6:10].sum(0)])
        v = (st1[:, 10:18] + st2[:, 10:18]).reshape(-1)

        loss_c = (
            0.5 * (n_t.sum() ** 2 - (n_t**2).sum())
            - ((v**2).sum() - n_t.sum())
            + NPAIR * C
        ) / (C * C)
        loss_s = (
            0.5 * ((M**2).sum() - sumA)
            - ((R**2).sum() - sumS)
            + NPAIR * W
        ) / (W * W)
        total += loss_c + loss_s

    return np.float32(total)


# revision 12
# speedup vs baseline: 1.2478x; 1.2478x over previous
"""AttMatrixCov loss kernel for 8 Trainium2 NeuronCores.

Math
----
Reference:
    loss = sum_{a, i<j} mean((attc[a,i] outer attc[a,j] - I_C)^2)
         + sum_{a, i<j} mean((atts[a,i]^T atts[a,j] - I_W)^2)

Using sum_{c,d}(x_c y_d - d_cd)^2 = |x|^2 |y|^2 - 2 x.y + C and
|S_i^T S_j|_F^2 = <A_i, A_j> with A_t = S_t S_t^T, every pairwise sum
collapses via sum_{i<j} <u_i, u_j> = 1/2 (|sum_t u_t|^2 - sum_t |u_t|^2):

    loss_c[a] = ( 1/2((sum_t n_t)^2 - sum_t n_t^2)
                - (|sum_t c_t|^2 - sum_t n_t) + P*C ) / C^2,  n_t = |attc[a,t]|^2
    loss_s[a] = ( 1/2(|M_a|_F^2 - sum_t |A_t|_F^2)
                - (|R_a|_F^2 - sum_t |S_t|_F^2) + P*W ) / W^2,
                M_a = sum_t A_t,  R_a = sum_t S_t,  P = 28 pairs.

Sharding: 8 cores = (natt=4) x (ntemp halves=2). Each core loads its 4
atts slices (1 MB f32) plus a bf16 copy (0.5 MB) that feeds the PE:
DMA-transpose loads give S^T directly, so the PE only runs the 16
A_t = S_t S_t^T matmuls (bf16 operands, f32 PSUM accumulate; measured
end-to-end rel err ~1e-5). R, |S|^2 and the channel branch stay exact
f32. Host combines per-core partials (cross-half |M1+M2|^2 etc.).
"""

import numpy as np

NATT, NTEMP, C = 4, 8, 1024
H, W = 256, 256
TL = NTEMP // 2          # ntemp slices per core
NPAIR = NTEMP * (NTEMP - 1) // 2
P = 128
N_CORES = 8

_nc_cache = None


def _build():
    import concourse.bacc as bacc
    import concourse.tile as tile
    from concourse import mybir

    f32 = mybir.dt.float32
    bf16 = mybir.dt.bfloat16
    nc = bacc.Bacc()
    s_in = nc.dram_tensor("s", [TL, H, W], f32, kind="ExternalInput")
    # host-side pre-transposed bf16 copy: sb[t, w, h] = atts[t, h, w]
    sb_in = nc.dram_tensor("sb", [TL, W, H], bf16, kind="ExternalInput")
    c_in = nc.dram_tensor("c", [TL, C], f32, kind="ExternalInput")
    m_out = nc.dram_tensor("m_out", [P, 2, H], f32, kind="ExternalOutput")
    r_out = nc.dram_tensor("r_out", [H, W], f32, kind="ExternalOutput")
    st_out = nc.dram_tensor("stats", [P, 18], f32, kind="ExternalOutput")

    with tile.TileContext(nc) as tc:
        with (
            tc.tile_pool(name="sall", bufs=1) as sall,
            tc.tile_pool(name="htp", bufs=4) as htp,
            tc.tile_pool(name="acc", bufs=1) as accp,
            tc.tile_pool(name="scr", bufs=2) as scr,
            tc.tile_pool(name="ps_a", bufs=2, space="PSUM") as ps_a,
        ):
            # ---- loads ----
            hts = []
            for t in range(TL):
                ht = htp.tile([P, 2, H], bf16)  # S^T: [p, wb, h]
                nc.sync.dma_start(
                    out=ht, in_=sb_in[t].rearrange("(wb p) h -> p wb h", p=P)
                )
                hts.append(ht)
            s_all = sall.tile([P, TL, 2, W], f32)  # [p, t, hb, w]
            for t in range(TL):
                nc.gpsimd.dma_start(
                    out=s_all[:, t],
                    in_=s_in[t].rearrange("(hb p) w -> p hb w", p=P),
                )
            c3 = sall.tile([P, TL, 8], f32)  # c3[p,t,f] = attc[t, p*8+f]
            nc.sync.dma_start(out=c3, in_=c_in.rearrange("t (p f) -> p t f", p=P))

            m_acc = accp.tile([P, 2, H], f32)
            r_acc = accp.tile([P, 2, W], f32)
            stats_a = accp.tile([P, TL], f32)   # ACT-written: |A_t|^2 partials
            stats_d = accp.tile([P, 14], f32)
            # stats_d cols: 0:2 |S|^2 half-partials, 2:6 n_t, 6:14 v

            # ---- channel branch (gpsimd, exact f32) ----
            csq = scr.tile([P, TL, 8], f32)
            nc.gpsimd.tensor_mul(csq, c3, c3)
            nc.vector.reduce_sum(
                stats_d[:, 2:6].rearrange("p (a b) -> p a b", b=1),
                csq,
                axis=mybir.AxisListType.X,
            )
            vtmp = scr.tile([P, 2, 8], f32)
            nc.gpsimd.tensor_add(vtmp, c3[:, 0:2, :], c3[:, 2:4, :])
            nc.gpsimd.tensor_add(stats_d[:, 6:14], vtmp[:, 0, :], vtmp[:, 1, :])

            # ---- A_t matmuls + M/sumA ----
            for t in range(TL):
                ht = hts[t]
                psa = ps_a.tile([P, 2, H], f32)
                for m in range(2):
                    for k in range(2):
                        nc.tensor.matmul(
                            psa[:, m, :],
                            lhsT=ht[:, k, m * P : (m + 1) * P],
                            rhs=ht[:, k, :],
                            start=(k == 0),
                            stop=(k == 1),
                        )
                a_scr = scr.tile([P, 2, H], f32, tag="ascr")
                nc.scalar.activation(
                    out=a_scr,
                    in_=psa,
                    func=mybir.ActivationFunctionType.Square,
                    accum_out=stats_a[:, t : t + 1],
                )
                if t == 0:
                    nc.vector.tensor_copy(m_acc, psa)
                else:
                    nc.vector.tensor_add(m_acc, m_acc, psa)

            # ---- R tree + |S|^2 (DVE, exact f32) ----
            r23 = scr.tile([P, 2, W], f32)
            nc.vector.tensor_add(r_acc, s_all[:, 0], s_all[:, 1])
            nc.vector.tensor_add(r23, s_all[:, 2], s_all[:, 3])
            nc.vector.tensor_add(r_acc, r_acc, r23)
            for hhalf in range(2):
                ssq = scr.tile([P, 2, 2, W], f32, tag="ssq")
                nc.vector.scalar_tensor_tensor(
                    out=ssq,
                    in0=s_all[:, 2 * hhalf : 2 * hhalf + 2],
                    scalar=1.0,
                    in1=s_all[:, 2 * hhalf : 2 * hhalf + 2],
                    op0=mybir.AluOpType.bypass,
                    op1=mybir.AluOpType.mult,
                    accum_out=stats_d[:, hhalf : hhalf + 1],
                )

            # ---- outputs ----
            nc.sync.dma_start(out=m_out[:, :, :], in_=m_acc)
            nc.sync.dma_start(
                out=r_out.rearrange("(hb p) w -> p hb w", p=P), in_=r_acc
            )
            nc.sync.dma_start(out=st_out[:, 0:4], in_=stats_a)
            nc.sync.dma_start(out=st_out[:, 4:18], in_=stats_d)
    nc.finalize()
    return nc


last_results = None


def _ensure_ntff_hook():
    """Register the axon NTFF profile hook if the image's antenv lacks it.

    Only matters when BASS_TRACE=1; harmless otherwise."""
    import sys
    import types

    try:
        import antenv.axon_hooks  # noqa: F401

        return
    except ImportError:
        pass
    try:
        from trn_agent_boot.trn_boot import _ntff_profile_via_ctypes

        hook = _ntff_profile_via_ctypes("/opt/axon/libaxon_pjrt.so")
    except Exception:
        hook = None
    mod = types.ModuleType("antenv.axon_hooks")
    mod.get_axon_ntff_profile_hook = lambda: hook
    mod.set_axon_ntff_profile_hook = lambda h: None
    sys.modules["antenv.axon_hooks"] = mod


def kernel(attc: np.ndarray, atts: np.ndarray) -> np.ndarray:
    global _nc_cache, last_results
    _ensure_ntff_hook()
    import ml_dtypes
    from concourse.bass_utils import run_bass_kernel_spmd

    if _nc_cache is None:
        _nc_cache = _build()
    nc = _nc_cache

    in_maps = []
    for core in range(N_CORES):
        a, hhalf = core // 2, core % 2
        sl = slice(hhalf * TL, (hhalf + 1) * TL)
        s_shard = np.ascontiguousarray(atts[a, sl]).astype(np.float32)
        in_maps.append(
            {
                "s": s_shard,
                "sb": np.ascontiguousarray(
                    s_shard.transpose(0, 2, 1)
                ).astype(ml_dtypes.bfloat16),
                "c": np.ascontiguousarray(attc[a, sl]).astype(np.float32),
            }
        )

    res = run_bass_kernel_spmd(nc, in_maps, core_ids=list(range(N_CORES)))
    last_results = res
    outs = res.results

    total = 0.0
    for a in range(NATT):
        o1, o2 = outs[2 * a], outs[2 * a + 1]
        mo = o1["m_out"].astype(np.float64) + o2["m_out"].astype(np.float64)
        M = mo.transpose(1, 0, 2).reshape(H, H)  # rows m*128+p
        R = o1["r_out"].astype(np.float64) + o2["r_out"].astype(np.float64)
        st1 = o1["stats"].astype(np.float64)
        st2 = o2["stats"].astype(np.float64)
        sumA = st1[:, 0:4].sum() + st2[:, 0:4].sum()
        sumS = st1[:, 4:6].sum() + st2[:, 4:6].sum()
        n_t = np.concatenate([st1[:, 6:10].sum(0), st2[:, 6:10].sum(0)])
        v = (st1[:, 10:18] + st2[:, 10:18]).reshape(-1)

        loss_c = (
            0.5 * (n_t.sum() ** 2 - (n_t**2).sum())
            - ((v**2).sum() - n_t.sum())
            + NPAIR * C
        ) / (C * C)
        loss_s = (
            0.5 * ((M**2).sum() - sumA)
            - ((R**2).sum() - sumS)
            + NPAIR * W
        ) / (W * W)
        total += loss_c + loss_s

    return np.float32(total)


# revision 14
# speedup vs baseline: 1.2705x; 1.0182x over previous
"""AttMatrixCov loss kernel for 8 Trainium2 NeuronCores.

Math
----
Reference:
    loss = sum_{a, i<j} mean((attc[a,i] outer attc[a,j] - I_C)^2)
         + sum_{a, i<j} mean((atts[a,i]^T atts[a,j] - I_W)^2)

Using sum_{c,d}(x_c y_d - d_cd)^2 = |x|^2 |y|^2 - 2 x.y + C and
|S_i^T S_j|_F^2 = <A_i, A_j> with A_t = S_t S_t^T, every pairwise sum
collapses via sum_{i<j} <u_i, u_j> = 1/2 (|sum_t u_t|^2 - sum_t |u_t|^2):

    loss_c[a] = ( 1/2((sum_t n_t)^2 - sum_t n_t^2)
                - (|sum_t c_t|^2 - sum_t n_t) + P*C ) / C^2,  n_t = |attc[a,t]|^2
    loss_s[a] = ( 1/2(|M_a|_F^2 - sum_t |A_t|_F^2)
                - (|R_a|_F^2 - sum_t |S_t|_F^2) + P*W ) / W^2,
                M_a = sum_t A_t,  R_a = sum_t S_t,  P = 28 pairs.

Sharding: 8 cores = (natt=4) x (ntemp halves=2). Each core loads its 4
atts slices (1 MB f32) plus a host-pre-transposed bf16 copy (0.5 MB)
that feeds the PE directly (no on-device transpose): the PE runs only
the 16 A_t = S_t S_t^T matmuls (bf16 operands, f32 PSUM accumulate;
measured end-to-end rel err ~1e-5). R, |S|^2 and the channel branch
stay exact f32. Host combines per-core partials.

Layout notes: the bf16 sidecar is stored [w, h]; SBUF tile ht[p,g,h]
holds row w = 2p+g so each partition reads 1 KB contiguous (big DMA
packets). Contraction over w is order-invariant, so the two matmul
K-chunks are just the g=0 / g=1 slices.
"""

import numpy as np

NATT, NTEMP, C = 4, 8, 1024
H, W = 256, 256
TL = NTEMP // 2          # ntemp slices per core
NPAIR = NTEMP * (NTEMP - 1) // 2
P = 128
N_CORES = 8

_nc_cache = None


def _build():
    import concourse.bacc as bacc
    import concourse.tile as tile
    from concourse import mybir

    f32 = mybir.dt.float32
    bf16 = mybir.dt.bfloat16
    nc = bacc.Bacc()
    s_in = nc.dram_tensor("s", [TL, H, W], f32, kind="ExternalInput")
    # host-side pre-transposed bf16 copy: sb[t, w, h] = atts[t, h, w]
    sb_in = nc.dram_tensor("sb", [TL, W, H], bf16, kind="ExternalInput")
    c_in = nc.dram_tensor("c", [TL, C], f32, kind="ExternalInput")
    m_out = nc.dram_tensor("m_out", [P, 2, H], f32, kind="ExternalOutput")
    r_out = nc.dram_tensor("r_out", [H, W], f32, kind="ExternalOutput")
    st_out = nc.dram_tensor("stats", [P, 18], f32, kind="ExternalOutput")

    with tile.TileContext(nc) as tc:
        with (
            tc.tile_pool(name="sall", bufs=1) as sall,
            tc.tile_pool(name="htp", bufs=4) as htp,
            tc.tile_pool(name="acc", bufs=1) as accp,
            tc.tile_pool(name="scr", bufs=2) as scr,
            tc.tile_pool(name="ps_a", bufs=2, space="PSUM") as ps_a,
        ):
            # ---- loads: ht (PE-critical) split across both HWDGE engines
            hts = []
            for t in range(TL):
                ht = htp.tile([P, 2, H], bf16)  # [p, g, h] = S^T[2p+g, h]
                eng = nc.sync if t < 2 else nc.scalar
                eng.dma_start(
                    out=ht, in_=sb_in[t].rearrange("(p g) h -> p g h", g=2)
                )
                hts.append(ht)
            s_all = sall.tile([P, TL, 2, W], f32)  # [p, t, hb, w]
            for t in range(TL):
                eng = nc.sync if t < 2 else nc.scalar
                eng.dma_start(
                    out=s_all[:, t],
                    in_=s_in[t].rearrange("(hb p) w -> p hb w", p=P),
                )
            c3 = sall.tile([P, TL, 8], f32)  # c3[p,t,f] = attc[t, p*8+f]
            nc.gpsimd.dma_start(
                out=c3, in_=c_in.rearrange("t (p f) -> p t f", p=P)
            )

            m_acc = accp.tile([P, 2, H], f32)
            m01 = scr.tile([P, 2, H], f32, tag="mtree")
            r_acc = accp.tile([P, 2, W], f32)
            r4 = scr.tile([P, 2, 2, W], f32, tag="rtree")
            stats_a = accp.tile([P, TL], f32)   # ACT: |A_t|^2 partials
            stats_d = accp.tile([P, 14], f32)
            # stats_d cols: 0:2 |S|^2 half-partials, 2:6 n_t, 6:14 v

            # ---- A_t matmuls + per-t squares ----
            psas = []
            for t in range(TL):
                ht = hts[t]
                psa = ps_a.tile([P, 2, H], f32)
                for m in range(2):
                    for k in range(2):
                        nc.tensor.matmul(
                            psa[:, m, :],
                            lhsT=ht[:, k, m * P : (m + 1) * P],
                            rhs=ht[:, k, :],
                            start=(k == 0),
                            stop=(k == 1),
                        )
                a_scr = scr.tile([P, 2, H], f32, tag="ascr")
                nc.scalar.activation(
                    out=a_scr,
                    in_=psa,
                    func=mybir.ActivationFunctionType.Square,
                    accum_out=stats_a[:, t : t + 1],
                )
                psas.append(psa)
                if t == 0:
                    nc.vector.tensor_copy(m_acc, psa)
                else:
                    nc.vector.tensor_add(m_acc, m_acc, psa)

            # ---- R tree + |S|^2 (DVE, exact f32) ----
            nc.vector.tensor_add(r4, s_all[:, 0:2], s_all[:, 2:4])
            nc.vector.tensor_add(r_acc, r4[:, 0], r4[:, 1])
            for hhalf in range(2):
                ssq = scr.tile([P, 2, 2, W], f32, tag="ssq")
                nc.vector.scalar_tensor_tensor(
                    out=ssq,
                    in0=s_all[:, 2 * hhalf : 2 * hhalf + 2],
                    scalar=1.0,
                    in1=s_all[:, 2 * hhalf : 2 * hhalf + 2],
                    op0=mybir.AluOpType.bypass,
                    op1=mybir.AluOpType.mult,
                    accum_out=stats_d[:, hhalf : hhalf + 1],
                )

            # ---- channel branch (gpsimd + one DVE reduce, exact f32) ----
            csq = scr.tile([P, TL, 8], f32)
            nc.gpsimd.tensor_mul(csq, c3, c3)
            nc.vector.reduce_sum(
                stats_d[:, 2:6].rearrange("p (a b) -> p a b", b=1),
                csq,
                axis=mybir.AxisListType.X,
            )
            vtmp = scr.tile([P, 2, 8], f32)
            nc.gpsimd.tensor_add(vtmp, c3[:, 0:2, :], c3[:, 2:4, :])
            nc.gpsimd.tensor_add(stats_d[:, 6:14], vtmp[:, 0, :], vtmp[:, 1, :])

            # ---- outputs: big ones on SWDGE (gpsimd), small on sync ----
            nc.gpsimd.dma_start(out=m_out[:, :, :], in_=m_acc)
            nc.gpsimd.dma_start(
                out=r_out.rearrange("(hb p) w -> p hb w", p=P), in_=r_acc
            )
            nc.sync.dma_start(out=st_out[:, 0:4], in_=stats_a)
            nc.sync.dma_start(out=st_out[:, 4:18], in_=stats_d)
    nc.finalize()
    return nc


last_results = None


def _ensure_ntff_hook():
    """Register the axon NTFF profile hook if the image's antenv lacks it.

    Only matters when BASS_TRACE=1; harmless otherwise."""
    import sys
    import types

    try:
        import antenv.axon_hooks  # noqa: F401

        return
    except ImportError:
        pass
    try:
        from trn_agent_boot.trn_boot import _ntff_profile_via_ctypes

        hook = _ntff_profile_via_ctypes("/opt/axon/libaxon_pjrt.so")
    except Exception:
        hook = None
    mod = types.ModuleType("antenv.axon_hooks")
    mod.get_axon_ntff_profile_hook = lambda: hook
    mod.set_axon_ntff_profile_hook = lambda h: None
    sys.modules["antenv.axon_hooks"] = mod


def kernel(attc: np.ndarray, atts: np.ndarray) -> np.ndarray:
    global _nc_cache, last_results
    _ensure_ntff_hook()
    import ml_dtypes
    from concourse.bass_utils import run_bass_kernel_spmd

    if _nc_cache is None:
        _nc_cache = _build()
    nc = _nc_cache

    in_maps = []
    for core in range(N_CORES):
        a, hhalf = core // 2, core % 2
        sl = slice(hhalf * TL, (hhalf + 1) * TL)
        s_shard = np.ascontiguousarray(atts[a, sl]).astype(np.float32)
        in_maps.append(
            {
                "s": s_shard,
                "sb": np.ascontiguousarray(
                    s_shard.transpose(0, 2, 1)
                ).astype(ml_dtypes.bfloat16),
                "c": np.ascontiguousarray(attc[a, sl]).astype(np.float32),
            }
        )

    res = run_bass_kernel_spmd(nc, in_maps, core_ids=list(range(N_CORES)))
    last_results = res
    outs = res.results

    total = 0.0
    for a in range(NATT):
        o1, o2 = outs[2 * a], outs[2 * a + 1]
        mo = o1["m_out"].astype(np.float64) + o2["m_out"].astype(np.float64)
        M = mo.transpose(1, 0, 2).reshape(H, H)  # rows m*128+p
        R = o1["r_out"].astype(np.float64) + o2["r_out"].astype(np.float64)
        st1 = o1["stats"].astype(np.float64)
        st2 = o2["stats"].astype(np.float64)
        sumA = st1[:, 0:4].sum() + st2[:, 0:4].sum()
        sumS = st1[:, 4:6].sum() + st2[:, 4:6].sum()
        n_t = np.concatenate([st1[:, 6:10].sum(0), st2[:, 6:10].sum(0)])
        v = (st1[:, 10:18] + st2[:, 10:18]).reshape(-1)

        loss_c = (
            0.5 * (n_t.sum() ** 2 - (n_t**2).sum())
            - ((v**2).sum() - n_t.sum())
            + NPAIR * C
        ) / (C * C)
        loss_s = (
            0.5 * ((M**2).sum() - sumA)
            - ((R**2).sum() - sumS)
            + NPAIR * W
        ) / (W * W)
        total += loss_c + loss_s

    return np.float32(total)


# revision 15
# speedup vs baseline: 1.5782x; 1.2422x over previous
"""AttMatrixCov loss kernel for 8 Trainium2 NeuronCores.

Math
----
Reference:
    loss = sum_{a, i<j} mean((attc[a,i] outer attc[a,j] - I_C)^2)
         + sum_{a, i<j} mean((atts[a,i]^T atts[a,j] - I_W)^2)

Using sum_{c,d}(x_c y_d - d_cd)^2 = |x|^2 |y|^2 - 2 x.y + C and
|S_i^T S_j|_F^2 = <A_i, A_j> with A_t = S_t S_t^T, every pairwise sum
collapses via sum_{i<j} <u_i, u_j> = 1/2 (|sum_t u_t|^2 - sum_t |u_t|^2):

    loss_c[a] = ( 1/2((sum_t n_t)^2 - sum_t n_t^2)
                - (|sum_t c_t|^2 - sum_t n_t) + P*C ) / C^2,  n_t = |attc[a,t]|^2
    loss_s[a] = ( 1/2(|M_a|_F^2 - sum_t |A_t|_F^2)
                - (|R_a|_F^2 - sum_t |S_t|_F^2) + P*W ) / W^2,
                M_a = sum_t A_t,  R_a = sum_t S_t,  P = 28 pairs.

Sharding: 8 cores = (natt=4) x (ntemp halves=2). Each core loads its 4
atts slices (1 MB f32) plus a host-pre-transposed bf16 copy (0.5 MB)
that feeds the PE directly: the PE runs only the 16 A_t = S_t S_t^T
matmuls (bf16 operands, f32 PSUM accumulate; measured end-to-end rel
err ~1e-5). R and the channel branch stay exact f32; |S|^2 uses the
bf16 copy (contributes ~1e-9 rel). Host combines per-core partials.

Layout notes: all inputs are host-shuffled into [partition, ...]
layouts whose per-partition data is contiguous in DRAM (4-8 KB runs),
because DMA throughput here is packet-rate-bound (~8 pkts/us/engine).
The bf16 sidecar holds S^T rows w = 2p+g in partition p; contraction
over w is order-invariant so the matmul K-chunks are the g slices.
A block of dummy warm-up matmuls runs during the (otherwise idle)
kernel prologue to lift the PE HAM clock gate from 1.2 to 2.4 GHz.
"""

import numpy as np

NATT, NTEMP, C = 4, 8, 1024
H, W = 256, 256
TL = NTEMP // 2          # ntemp slices per core
NPAIR = NTEMP * (NTEMP - 1) // 2
P = 128
N_CORES = 8
WARM_MM = 10

_nc_cache = None


def _build():
    import concourse.bacc as bacc
    import concourse.tile as tile
    from concourse import mybir

    f32 = mybir.dt.float32
    bf16 = mybir.dt.bfloat16
    nc = bacc.Bacc()
    # host-shuffled: s[p, t, hb, w] = atts[t, hb*128+p, w]
    s_in = nc.dram_tensor("s", [P, TL, 2, W], f32, kind="ExternalInput")
    # host-transposed bf16: sb[p, t, g, h] = atts[t, h, 2p+g]
    sb_in = nc.dram_tensor("sb", [P, TL, 2, H], bf16, kind="ExternalInput")
    # c[p, t, f] = attc[t, p*8+f]
    c_in = nc.dram_tensor("c", [P, TL, 8], f32, kind="ExternalInput")
    m_out = nc.dram_tensor("m_out", [P, 2, H], f32, kind="ExternalOutput")
    r_out = nc.dram_tensor("r_out", [P, 2, W], f32, kind="ExternalOutput")
    st_out = nc.dram_tensor("stats", [P, 18], f32, kind="ExternalOutput")

    with tile.TileContext(nc) as tc:
        with (
            tc.tile_pool(name="sall", bufs=1) as sall,
            tc.tile_pool(name="acc", bufs=1) as accp,
            tc.tile_pool(name="scr", bufs=2) as scr,
            tc.tile_pool(name="ps_a", bufs=2, space="PSUM") as ps_a,
            tc.tile_pool(name="ps_w", bufs=1, space="PSUM") as ps_w,
        ):
            # ---- loads (one contiguous DMA per tensor) ----
            ht_all = sall.tile([P, TL, 2, H], bf16)
            nc.sync.dma_start(out=ht_all, in_=sb_in[:, :])
            s_all = sall.tile([P, TL, 2, W], f32)
            nc.scalar.dma_start(out=s_all, in_=s_in[:, :])
            c3 = sall.tile([P, TL, 8], f32)
            nc.sync.dma_start(out=c3, in_=c_in[:, :])

            # ---- PE warm-up (lifts HAM clock gate during prologue) ----
            wtile = sall.tile([P, 512], bf16)
            nc.vector.memset(wtile, 0.0)
            pwarm = ps_w.tile([P, 512], f32)
            for i in range(WARM_MM):
                nc.tensor.matmul(
                    pwarm[:, :],
                    lhsT=wtile[:, 0:P],
                    rhs=wtile[:, :],
                    start=(i == 0),
                    stop=(i == WARM_MM - 1),
                )

            m_acc = accp.tile([P, 2, H], f32)
            r_acc = accp.tile([P, 2, W], f32)
            r4 = scr.tile([P, 2, 2, W], f32, tag="rtree")
            stats_a = accp.tile([P, TL], f32)   # ACT: |A_t|^2 partials
            stats_d = accp.tile([P, 14], f32)
            # stats_d cols: 0 |S|^2 partial (bf16), 2:6 n_t, 6:14 v

            # ---- A_t matmuls + per-t squares + M chain ----
            for t in range(TL):
                psa = ps_a.tile([P, 2, H], f32)
                for m in range(2):
                    for k in range(2):
                        nc.tensor.matmul(
                            psa[:, m, :],
                            lhsT=ht_all[:, t, k, m * P : (m + 1) * P],
                            rhs=ht_all[:, t, k, :],
                            start=(k == 0),
                            stop=(k == 1),
                        )
                a_scr = scr.tile([P, 2, H], f32, tag="ascr")
                nc.scalar.activation(
                    out=a_scr,
                    in_=psa,
                    func=mybir.ActivationFunctionType.Square,
                    accum_out=stats_a[:, t : t + 1],
                )
                if t == 0:
                    nc.vector.tensor_copy(m_acc, psa)
                else:
                    nc.vector.tensor_add(m_acc, m_acc, psa)

            # ---- R tree (exact f32) + |S|^2 (bf16 copy) ----
            nc.vector.tensor_add(r4, s_all[:, 0:2], s_all[:, 2:4])
            nc.vector.tensor_add(r_acc, r4[:, 0], r4[:, 1])
            ssq = sall.tile([P, TL, 2, H], f32)
            nc.vector.scalar_tensor_tensor(
                out=ssq,
                in0=ht_all,
                scalar=1.0,
                in1=ht_all,
                op0=mybir.AluOpType.bypass,
                op1=mybir.AluOpType.mult,
                accum_out=stats_d[:, 0:1],
            )

            # ---- channel branch (gpsimd + one DVE reduce, exact f32) ----
            csq = scr.tile([P, TL, 8], f32)
            nc.gpsimd.tensor_mul(csq, c3, c3)
            nc.vector.reduce_sum(
                stats_d[:, 2:6].rearrange("p (a b) -> p a b", b=1),
                csq,
                axis=mybir.AxisListType.X,
            )
            vtmp = scr.tile([P, 2, 8], f32)
            nc.gpsimd.tensor_add(vtmp, c3[:, 0:2, :], c3[:, 2:4, :])
            nc.gpsimd.tensor_add(stats_d[:, 6:14], vtmp[:, 0, :], vtmp[:, 1, :])

            # ---- outputs ----
            nc.sync.dma_start(out=m_out[:, :, :], in_=m_acc)
            nc.scalar.dma_start(out=r_out[:, :, :], in_=r_acc)
            nc.sync.dma_start(out=st_out[:, 0:4], in_=stats_a)
            nc.sync.dma_start(out=st_out[:, 4:18], in_=stats_d)
    nc.finalize()
    return nc


last_results = None


def _ensure_ntff_hook():
    """Register the axon NTFF profile hook if the image's antenv lacks it.

    Only matters when BASS_TRACE=1; harmless otherwise."""
    import sys
    import types

    try:
        import antenv.axon_hooks  # noqa: F401

        return
    except ImportError:
        pass
    try:
        from trn_agent_boot.trn_boot import _ntff_profile_via_ctypes

        hook = _ntff_profile_via_ctypes("/opt/axon/libaxon_pjrt.so")
    except Exception:
        hook = None
    mod = types.ModuleType("antenv.axon_hooks")
    mod.get_axon_ntff_profile_hook = lambda: hook
    mod.set_axon_ntff_profile_hook = lambda h: None
    sys.modules["antenv.axon_hooks"] = mod


def kernel(attc: np.ndarray, atts: np.ndarray) -> np.ndarray:
    global _nc_cache, last_results
    _ensure_ntff_hook()
    import ml_dtypes
    from concourse.bass_utils import run_bass_kernel_spmd

    if _nc_cache is None:
        _nc_cache = _build()
    nc = _nc_cache

    in_maps = []
    for core in range(N_CORES):
        a, hhalf = core // 2, core % 2
        sl = slice(hhalf * TL, (hhalf + 1) * TL)
        x = np.ascontiguousarray(atts[a, sl]).astype(np.float32)  # [4,256,256]
        # s[p, t, hb, w] = x[t, hb*128+p, w]
        s_host = np.ascontiguousarray(
            x.reshape(TL, 2, P, W).transpose(2, 0, 1, 3)
        )
        # sb[p, t, g, h] = x[t, h, 2p+g]
        sb_host = np.ascontiguousarray(
            x.transpose(2, 0, 1).reshape(P, 2, TL, H).transpose(0, 2, 1, 3)
        ).astype(ml_dtypes.bfloat16)
        cc = np.ascontiguousarray(attc[a, sl]).astype(np.float32)  # [4,1024]
        c_host = np.ascontiguousarray(
            cc.reshape(TL, P, 8).transpose(1, 0, 2)
        )
        in_maps.append({"s": s_host, "sb": sb_host, "c": c_host})

    res = run_bass_kernel_spmd(nc, in_maps, core_ids=list(range(N_CORES)))
    last_results = res
    outs = res.results

    total = 0.0
    for a in range(NATT):
        o1, o2 = outs[2 * a], outs[2 * a + 1]
        mo = o1["m_out"].astype(np.float64) + o2["m_out"].astype(np.float64)
        M = mo.transpose(1, 0, 2).reshape(H, H)  # rows m*128+p
        R = o1["r_out"].astype(np.float64) + o2["r_out"].astype(np.float64)
        st1 = o1["stats"].astype(np.float64)
        st2 = o2["stats"].astype(np.float64)
        sumA = st1[:, 0:4].sum() + st2[:, 0:4].sum()
        sumS = st1[:, 4].sum() + st2[:, 4].sum()
        n_t = np.concatenate([st1[:, 6:10].sum(0), st2[:, 6:10].sum(0)])
        v = (st1[:, 10:18] + st2[:, 10:18]).reshape(-1)

        loss_c = (
            0.5 * (n_t.sum() ** 2 - (n_t**2).sum())
            - ((v**2).sum() - n_t.sum())
            + NPAIR * C
        ) / (C * C)
        loss_s = (
            0.5 * ((M**2).sum() - sumA)
            - ((R**2).sum() - sumS)
            + NPAIR * W
        ) / (W * W)
        total += loss_c + loss_s

    return np.float32(total)
